# revision 15
# baseline (speedup 1.0000x reference)
"""Trainium2 Bass kernel for MatrixGATVAE (2-layer GATv2 encoder), 8-core SPMD.

kernel(**inputs): FULL numpy inputs -> FULL [20000, 128] f32 output.
Sharding: nodes + in-edges by destination across 8 cores; weights replicated;
src-side tables all-gathered; per-block batched dma_gather for src edge rows.

Edge stage (v2):
- e = att.leaky_relu(z) computed directly: w = Lrelu(z, 0.2) on ACT, then two
  even-aligned range reduces (+ odd-Fp straggler column fix) on DVE.
- dst-side rows (z = ga + xra[dst]): per-block 128 unique rows; first TBS
  sub-blocks use dma_gather (POOL), the rest are expanded on PE via the
  transposed one-hot (SaT x XRB -> PSUM) and added to ga on DVE.
- Aggregation: one-hot scatter matmuls (Sa scaled by AA on ACT) as before.
- Conv1d + flatten folded into projection weights; tables pre-scaled by |att|;
  BatchNorm folded into layer-2 projection weights on device.
"""

import os
import sys

import numpy as np

sys.path.insert(0, "/opt/trn_rl_repo")

import concourse.bass as bass
import concourse.bacc as bacc
import concourse.mybir as mybir
import concourse.tile as tile
from concourse.bass_utils import run_bass_kernel_spmd

BF16 = mybir.dt.bfloat16
F32 = mybir.dt.float32
FP8 = mybir.dt.float8e4
I16 = mybir.dt.int16
AF = mybir.ActivationFunctionType
ALU = mybir.AluOpType
AX = mybir.AxisListType

N = 20000
NCORES = 8
NLOC = N // NCORES            # 2500
NPAD = 2560
NB = NPAD // 128              # 20
KW, TPOS, COUT = 24, 32, 64
F0 = 768
H = 512
L = 128
KC0 = F0 // 128               # 6
KC1 = H // 128                # 4
BN_EPS = 1e-5
NQUEUES = 2
AG_CHUNKS = 1

_cache = {}


def _bf16(a):
    import ml_dtypes
    return np.ascontiguousarray(np.asarray(a, dtype=np.float32)).astype(ml_dtypes.bfloat16)


def build_program(TB, TBS1, TBS2, Fp1, Fp2):
    NI = TB * 128
    TBP1, TBP2 = TB - TBS1, TB - TBS2
    nc = bacc.Bacc(num_devices=NCORES, num_swdge_queues=NQUEUES)

    def dp(name, shape, dtype, isOutput=False):
        return nc.declare_dram_parameter(name, shape, dtype, isOutput)

    xT = dp("xT", [128, KC0, NPAD], BF16)
    Wl = dp("Wl", [128, KC0, H], BF16)
    Wra = dp("Wra", [128, KC0, H], BF16)
    beff_l = dp("beff_l", [1, H], BF16)
    beff_ra = dp("beff_ra", [1, H], BF16)
    bias1b = dp("bias1b", [128, H], BF16)
    rcatt1 = dp("rcatt1", [128, H], BF16)
    W2l = dp("W2l", [128, KC1, L], BF16)
    W2ra = dp("W2ra", [128, KC1, L], BF16)
    b2l = dp("b2l", [1, L], BF16)
    b2ra = dp("b2ra", [1, L], BF16)
    rcatt2 = dp("rcatt2", [128, L], BF16)
    bias2b = dp("bias2b", [128, L], F32)
    gamma_c = dp("gamma_c", [128, KC1], F32)
    beta_c = dp("beta_c", [128, KC1], F32)
    maskl = dp("maskl", [128, 1], F32)
    ident = dp("ident", [128, 128], BF16)
    isrc16 = dp("isrc16", [128, NB, NI // 16], I16)
    idst16 = dp("idst16", [128, NB, max(TBS1, 1) * 8], I16)
    sabase = dp("sabase", [128, NB, TB, 128], FP8)
    sabT1 = dp("sabT1", [128, NB, max(TBP1, 1), 128], FP8)
    sabT2 = dp("sabT2", [128, NB, max(TBP2, 1), 128], FP8)
    mu_out = dp("mu", [NPAD, L], F32, isOutput=True)

    XLA_loc = nc.dram_tensor("XLA_loc", [NPAD, H], BF16)
    XLAg = nc.dram_tensor("XLAg", [N, H], BF16, addr_space="Shared")
    XRAT1 = nc.dram_tensor("XRAT1", [NPAD, H], BF16)
    H1Dc = [nc.dram_tensor(f"H1Dc{c}", [NPAD, 128], BF16) for c in range(KC1)]
    XLA2_loc = nc.dram_tensor("XLA2_loc", [NPAD, L], BF16)
    XLA2g = nc.dram_tensor("XLA2g", [N, L], BF16, addr_space="Shared")
    XRAT2 = nc.dram_tensor("XRAT2", [NPAD, L], BF16)
    ST_loc = nc.dram_tensor("ST_loc", [1, 2 * H], F32)
    ST_red = nc.dram_tensor("ST_red", [1, 2 * H], F32, addr_space="Shared")

    grp = [list(range(NCORES))]

    with tile.TileContext(nc) as tc:
        with (
            tc.tile_pool(name="const", bufs=1) as cpool,
            tc.tile_pool(name="zsrc", bufs=3) as zs_pool,
            tc.tile_pool(name="zdst", bufs=2) as zd_pool,
            tc.tile_pool(name="lw", bufs=1) as w_pool,
            tc.tile_pool(name="sab", bufs=2) as sab_pool,
            tc.tile_pool(name="work", bufs=3) as work,
            tc.tile_pool(name="blk", bufs=2) as blk,
            tc.tile_pool(name="acc", bufs=2, space="PSUM") as ps,
            tc.tile_pool(name="psS", bufs=1, space="PSUM") as ps_s,
            tc.tile_pool(name="psZ", bufs=2, space="PSUM") as ps_z,
        ):
            def load(tag, dram):
                t = cpool.tile(dram.shape, dram.dtype, tag=tag)
                nc.sync.dma_start(t[:], dram[:])
                return t

            Wl_s = load("Wl", Wl)
            Wra_s = load("Wra", Wra)
            beffl_s = load("beffl", beff_l)
            beffra_s = load("beffra", beff_ra)
            bias1b_s = load("bias1b", bias1b)
            rcatt1_s = load("rcatt1", rcatt1)
            W2l_s = load("W2l", W2l)
            W2ra_s = load("W2ra", W2ra)
            b2l_s = load("b2l", b2l)
            b2ra_s = load("b2ra", b2ra)
            rcatt2_s = load("rcatt2", rcatt2)
            bias2b_s = load("bias2b", bias2b)
            gamma_s = load("gamma", gamma_c)
            beta_s = load("beta", beta_c)
            maskl_s = load("maskl", maskl)
            ident_s = load("ident", ident)
            idst_s = load("idst16", idst16)

            ones_col = cpool.tile([128, 1], BF16, tag="ones_col")
            nc.vector.memset(ones_col[:], 1.0)
            one_row = cpool.tile([1, 128], BF16, tag="one_row")
            nc.vector.memset(one_row[:], 1.0)

            W2fl = cpool.tile([128, KC1, L], BF16, tag="W2fl")
            W2fra = cpool.tile([128, KC1, L], BF16, tag="W2fra")
            b2e = cpool.tile([1, L], BF16, tag="b2e")
            b2era = cpool.tile([1, L], BF16, tag="b2era")

            # ============ layer-1 projections -> tables ============
            # AllGather is chunked so transfers overlap the projection loop.
            assert NB % AG_CHUNKS == 0 and NLOC % AG_CHUNKS == 0
            bpc = NB // AG_CHUNKS
            rpc = NLOC // AG_CHUNKS
            for b in range(NB):
                sl = slice(b * 128, (b + 1) * 128)
                pl = ps.tile([128, H], F32, tag="acc")
                pra = ps.tile([128, H], F32, tag="acc")
                xTb = blk.tile([128, KC0, 128], BF16, tag="xTb")
                nc.sync.dma_start(xTb[:], xT[:, :, sl])
                for c in range(KC0):
                    lhsT = xTb[:, c, :]
                    nc.tensor.matmul(pl[:], lhsT, Wl_s[:, c, :], start=(c == 0), stop=False)
                    nc.tensor.matmul(pra[:], lhsT, Wra_s[:, c, :], start=(c == 0), stop=False)
                nc.tensor.matmul(pl[:], one_row[:1, :], beffl_s[:1, :], start=False, stop=True)
                nc.tensor.matmul(pra[:], one_row[:1, :], beffra_s[:1, :], start=False, stop=True)
                xl_sb = blk.tile([128, H], BF16, tag="xl")
                xra_sb = blk.tile([128, H], BF16, tag="xra")
                nc.scalar.activation(xl_sb[:], pl[:], AF.Copy)
                nc.scalar.activation(xra_sb[:], pra[:], AF.Copy)
                nc.sync.dma_start(XLA_loc[sl, :], xl_sb[:])
                nc.sync.dma_start(XRAT1[sl, :], xra_sb[:])
                if b % bpc == bpc - 1:
                    cchunk = b // bpc
                    rsl = slice(cchunk * rpc, (cchunk + 1) * rpc)
                    nc.gpsimd.collective_compute(
                        "AllGather", ALU.bypass, replica_groups=grp,
                        ins=[XLA_loc[rsl, :]],
                        outs=[XLAg[cchunk * NCORES * rpc:(cchunk + 1) * NCORES * rpc, :]])

            # ============ fused edge stage (logits + aggregation) ============
            # Software-pipelined; BPG blocks share one gather instruction to
            # amortize SWDGE fixed + completion overhead.
            def edge_stage(W, Fp, TBLg, XRAT_d, sabT, lay, post, BPG):
                NBG = NB // BPG
                TBG = TB * BPG
                def logit_phase(g):
                    isx = blk.tile([128, BPG * NI // 16], I16, tag="isx")
                    nc.sync.dma_start(
                        isx[:], isrc16[:, g * BPG:(g + 1) * BPG, :].rearrange(
                            "p b i -> p (b i)"))
                    ga = zs_pool.tile([128, TBG, W], BF16, tag="ga")
                    nc.gpsimd.dma_gather(
                        out_ap=ga[:, :, :], in_ap=TBLg[:, :], idxs_ap=isx[:, :],
                        num_idxs=BPG * NI, num_idxs_reg=BPG * NI, elem_size=W,
                        queue_num=0, single_packet=False)
                    z = zd_pool.tile([128, TBG, W], BF16, tag="z")
                    xrbs = []
                    for sb in range(BPG):
                        xrb = work.tile([128, W], BF16, tag=f"xrb{sb}")
                        nc.sync.dma_start(
                            xrb[:],
                            XRAT_d[(g * BPG + sb) * 128:(g * BPG + sb + 1) * 128, :])
                        xrbs.append(xrb)
                    sabT_s = sab_pool.tile([128, BPG, TB, 128], FP8, tag="sabT")
                    nc.sync.dma_start(sabT_s[:], sabT[:, g * BPG:(g + 1) * BPG, :, :])
                    w = w_pool.tile([128, TBG, W], BF16, tag="lw")
                    ee = work.tile([128, TBG], F32, tag=f"ee{lay}")
                    HG = (TBG + 1) // 2
                    for half in range(2):
                        t0, t1 = half * HG, min((half + 1) * HG, TBG)
                        t = t0
                        while t < t1:
                            tw = min(2, t1 - t)
                            pz = ps_z.tile([128, 2, W], F32, tag="pz")
                            for j in range(tw):
                                tt = t + j
                                nc.tensor.matmul(pz[:, j, :],
                                                 sabT_s[:, tt // TB, tt % TB, :],
                                                 xrbs[tt // TB][:],
                                                 start=True, stop=False)
                                nc.tensor.matmul(pz[:, j, :], ident_s[:],
                                                 ga[:, tt, :], start=False, stop=True)
                            nc.scalar.activation(z[:, t:t + tw, :], pz[:, 0:tw, :], AF.Copy)
                            t += tw
                        # exact leaky on this half: pos cols max(z,.2z);
                        # neg cols (tables pre-scaled by -0.2) min(z,5z)
                        nc.vector.scalar_tensor_tensor(
                            w[:, t0:t1, 0:Fp], z[:, t0:t1, 0:Fp], 0.2,
                            z[:, t0:t1, 0:Fp], ALU.mult, ALU.max)
                        nc.vector.scalar_tensor_tensor(
                            w[:, t0:t1, Fp:W], z[:, t0:t1, Fp:W], 5.0,
                            z[:, t0:t1, Fp:W], ALU.mult, ALU.min)
                        nc.vector.tensor_reduce(
                            ee[:, t0:t1], w[:, t0:t1, :], axis=AX.X, op=ALU.add)
                    SaB = sab_pool.tile([128, BPG, TB, 128], FP8, tag="SaB")
                    nc.sync.dma_start(SaB[:], sabase[:, g * BPG:(g + 1) * BPG, :, :])
                    return (g, ga, z, ee, SaB)

                def scatter_phase(st):
                    g, ga, z, ee, SaB = st
                    AA = work.tile([128, TBG], F32, tag=f"AA{lay}")
                    nc.scalar.activation(AA[:], ee[:], AF.Exp)
                    AAb = work.tile([128, TBG], BF16, tag=f"AAb{lay}")
                    nc.vector.tensor_copy(AAb[:], AA[:])
                    for sb in range(BPG):
                        pU = ps.tile([128, H], F32, tag="acc")
                        pS = ps_s.tile([128, 1], F32, tag="pS")
                        for k in range(TB):
                            t = sb * TB + k
                            nc.scalar.activation(
                                z[:, t, :], ga[:, t, :], AF.Copy,
                                scale=AA[:, t:t + 1])
                            nc.tensor.matmul(
                                pU[:, 0:W], SaB[:, sb, k, :], z[:, t, :],
                                start=(k == 0), stop=(k == TB - 1))
                            nc.tensor.matmul(
                                pS[:], SaB[:, sb, k, :], AAb[:, t:t + 1],
                                start=(k == 0), stop=(k == TB - 1))
                        post(g * BPG + sb, pU, pS)

                prev = None
                for g in range(NBG):
                    cur = logit_phase(g)
                    if prev is not None:
                        scatter_phase(prev)
                    prev = cur
                scatter_phase(prev)

            # ---- layer-1 edge stage -> h1 ----
            st_acc = cpool.tile([1, 2 * H], F32, tag="st_acc")
            nc.vector.memset(st_acc[:], 0.0)

            def post1(b, pU, pS):
                s_sb = blk.tile([128, 1], F32, tag="s1")
                nc.vector.tensor_scalar(s_sb[:], pS[:], 1e-16, None, ALU.add)
                r_sb = blk.tile([128, 1], F32, tag="r1")
                nc.vector.reciprocal(r_sb[:], s_sb[:])
                hsq = work.tile([128, 2, H], BF16, tag="hsq")
                h1 = hsq[:, 0, :]
                nc.vector.scalar_tensor_tensor(
                    h1[:], pU[:, 0:H], r_sb[:], rcatt1_s[:], ALU.mult, ALU.mult)
                nc.vector.tensor_tensor(h1[:], h1[:], bias1b_s[:], ALU.add)
                nc.vector.tensor_scalar(h1[:], h1[:], 0.0, None, ALU.max)
                if b == NB - 1:
                    nc.vector.tensor_scalar(h1[:], h1[:], maskl_s[:], None, ALU.mult)
                nc.vector.tensor_tensor(hsq[:, 1, :], h1[:], h1[:], ALU.mult)
                stp = ps_z.tile([1, 2 * H], F32, tag="pz")
                nc.tensor.matmul(stp[:, 0:H], ones_col[:], hsq[:, 0, :],
                                 start=True, stop=True)
                nc.tensor.matmul(stp[:, H:2 * H], ones_col[:], hsq[:, 1, :],
                                 start=True, stop=True)
                sts = blk.tile([1, 2 * H], F32, tag="sts")
                nc.scalar.activation(sts[:], stp[:], AF.Copy)
                nc.vector.tensor_tensor(st_acc[:], st_acc[:], sts[:], ALU.add)
                for c in range(KC1):
                    nc.sync.dma_start(
                        H1Dc[c][b * 128:(b + 1) * 128, :],
                        h1[:, c * 128:(c + 1) * 128])

            edge_stage(H, Fp1, XLAg, XRAT1, sabT1, 1, post1, BPG=1)

            # ---- h1T via DMA transpose; BN stats from h1T ----
            h1T = cpool.tile([128, KC1, NPAD], BF16, tag="h1T")
            for c in range(KC1):
                nc.sync.dma_start(h1T[:, c, :], H1Dc[c][:, :], transpose=True)

            nc.sync.dma_start(ST_loc[:, :], st_acc[:])
            nc.gpsimd.collective_compute(
                "AllReduce", ALU.add, replica_groups=grp,
                ins=[ST_loc[:, :]], outs=[ST_red[:, :]])
            str_sb = cpool.tile([128, 2, KC1], F32, tag="str_sb")
            nc.sync.dma_start(
                str_sb[:],
                ST_red[:, :].rearrange("o (a c p) -> (o p) a c", a=2, p=128))

            # ---- BN fold into layer-2 weights ----
            mean = cpool.tile([128, KC1], F32, tag="mean")
            var = cpool.tile([128, KC1], F32, tag="var")
            nc.vector.tensor_scalar(mean[:], str_sb[:, 0, :], 1.0 / N, None, ALU.mult)
            nc.vector.tensor_scalar(var[:], str_sb[:, 1, :], 1.0 / N, None, ALU.mult)
            m2 = cpool.tile([128, KC1], F32, tag="m2")
            nc.vector.tensor_tensor(m2[:], mean[:], mean[:], ALU.mult)
            nc.vector.tensor_tensor(var[:], var[:], m2[:], ALU.subtract)
            nc.vector.tensor_scalar(var[:], var[:], BN_EPS, None, ALU.add)
            sd = cpool.tile([128, KC1], F32, tag="sd")
            nc.scalar.activation(sd[:], var[:], AF.Sqrt)
            rsd = cpool.tile([128, KC1], F32, tag="rsd")
            nc.vector.reciprocal(rsd[:], sd[:])
            tmpn = cpool.tile([128, KC1], F32, tag="tmpn")
            nc.vector.tensor_tensor(tmpn[:], var[:], rsd[:], ALU.mult)
            nc.vector.tensor_tensor(sd[:], sd[:], tmpn[:], ALU.add)
            nc.vector.tensor_scalar(sd[:], sd[:], 0.5, None, ALU.mult)
            nc.vector.reciprocal(rsd[:], sd[:])
            scale = cpool.tile([128, KC1], F32, tag="scale")
            nc.vector.tensor_tensor(scale[:], gamma_s[:], rsd[:], ALU.mult)
            shift = cpool.tile([128, KC1], F32, tag="shift")
            nc.vector.tensor_tensor(shift[:], mean[:], scale[:], ALU.mult)
            nc.vector.tensor_tensor(shift[:], beta_s[:], shift[:], ALU.subtract)
            shift_bf = cpool.tile([128, KC1], BF16, tag="shift_bf")
            nc.vector.tensor_copy(shift_bf[:], shift[:])

            for c in range(KC1):
                nc.vector.tensor_scalar(
                    W2fl[:, c, :], W2l_s[:, c, :], scale[:, c:c + 1], None, ALU.mult)
                nc.vector.tensor_scalar(
                    W2fra[:, c, :], W2ra_s[:, c, :], scale[:, c:c + 1], None, ALU.mult)
            pb = ps.tile([128, H], F32, tag="acc")
            pbra = ps.tile([128, H], F32, tag="acc")
            for c in range(KC1):
                nc.tensor.matmul(pb[0:1, 0:L], shift_bf[:, c:c + 1], W2l_s[:, c, :],
                                 start=(c == 0), stop=False)
                nc.tensor.matmul(pbra[0:1, 0:L], shift_bf[:, c:c + 1], W2ra_s[:, c, :],
                                 start=(c == 0), stop=False)
            nc.tensor.matmul(pb[0:1, 0:L], one_row[:1, 0:1], b2l_s[:1, :], start=False, stop=True)
            nc.tensor.matmul(pbra[0:1, 0:L], one_row[:1, 0:1], b2ra_s[:1, :], start=False, stop=True)
            nc.vector.tensor_copy(b2e[:], pb[0:1, 0:L])
            nc.vector.tensor_copy(b2era[:], pbra[0:1, 0:L])

            # ---- layer-2 projections -> tables ----
            for b in range(NB):
                sl = slice(b * 128, (b + 1) * 128)
                p2 = ps.tile([128, H], F32, tag="acc")
                p2ra = ps.tile([128, H], F32, tag="acc")
                for c in range(KC1):
                    lhsT = h1T[:, c, sl]
                    nc.tensor.matmul(p2[:, 0:L], lhsT, W2fl[:, c, :], start=(c == 0), stop=False)
                    nc.tensor.matmul(p2ra[:, 0:L], lhsT, W2fra[:, c, :], start=(c == 0), stop=False)
                nc.tensor.matmul(p2[:, 0:L], one_row[:1, :], b2e[:1, :], start=False, stop=True)
                nc.tensor.matmul(p2ra[:, 0:L], one_row[:1, :], b2era[:1, :], start=False, stop=True)
                xl2_sb = blk.tile([128, L], BF16, tag="xl2")
                xra2_sb = blk.tile([128, L], BF16, tag="xra2")
                nc.scalar.activation(xl2_sb[:], p2[:, 0:L], AF.Copy)
                nc.scalar.activation(xra2_sb[:], p2ra[:, 0:L], AF.Copy)
                nc.sync.dma_start(XLA2_loc[sl, :], xl2_sb[:])
                nc.sync.dma_start(XRAT2[sl, :], xra2_sb[:])
                if b % bpc == bpc - 1:
                    cchunk = b // bpc
                    rsl = slice(cchunk * rpc, (cchunk + 1) * rpc)
                    nc.gpsimd.collective_compute(
                        "AllGather", ALU.bypass, replica_groups=grp,
                        ins=[XLA2_loc[rsl, :]],
                        outs=[XLA2g[cchunk * NCORES * rpc:(cchunk + 1) * NCORES * rpc, :]])

            # ---- layer-2 edge stage -> mu ----
            def post2(b, pU, pS):
                s_sb = blk.tile([128, 1], F32, tag="s2")
                nc.vector.tensor_scalar(s_sb[:], pS[:], 1e-16, None, ALU.add)
                r_sb = blk.tile([128, 1], F32, tag="r2")
                nc.vector.reciprocal(r_sb[:], s_sb[:])
                mu_sb = blk.tile([128, L], F32, tag="mu")
                nc.vector.scalar_tensor_tensor(
                    mu_sb[:], pU[:, 0:L], r_sb[:], rcatt2_s[:], ALU.mult, ALU.mult)
                nc.vector.tensor_tensor(mu_sb[:], mu_sb[:], bias2b_s[:], ALU.add)
                nc.sync.dma_start(mu_out[b * 128:(b + 1) * 128, :], mu_sb[:])

            edge_stage(L, Fp2, XLA2g, XRAT2, sabT2, 2, post2, BPG=2)

    nc.compile()
    return nc


def _prep_host(inputs):
    x = np.asarray(inputs["x"], dtype=np.float32)
    ei = np.asarray(inputs["edge_index"], dtype=np.int64)
    conv_w = np.asarray(inputs["conv_w"], dtype=np.float32)
    conv_b = np.asarray(inputs["conv_b"], dtype=np.float32)
    W1l = np.asarray(inputs["W1l"], dtype=np.float32)
    b1l = np.asarray(inputs["b1l"], dtype=np.float32)
    W1r = np.asarray(inputs["W1r"], dtype=np.float32)
    b1r = np.asarray(inputs["b1r"], dtype=np.float32)
    att1 = np.asarray(inputs["att1"], dtype=np.float32)
    bias1 = np.asarray(inputs["bias1"], dtype=np.float32)
    gamma = np.asarray(inputs["gamma"], dtype=np.float32)
    beta = np.asarray(inputs["beta"], dtype=np.float32)
    W2l = np.asarray(inputs["W2l"], dtype=np.float32)
    b2l = np.asarray(inputs["b2l"], dtype=np.float32)
    W2r = np.asarray(inputs["W2r"], dtype=np.float32)
    b2r = np.asarray(inputs["b2r"], dtype=np.float32)
    att2 = np.asarray(inputs["att2"], dtype=np.float32)
    bias2 = np.asarray(inputs["bias2"], dtype=np.float32)

    # conv fold: V[(k*32+t), j] = sum_o w[o,k] W[o*32+t, j]
    def fold(W):
        return np.einsum("ok,otj->ktj", conv_w,
                         W.reshape(COUT, TPOS, -1)).reshape(F0, -1)

    V_l, V_r = fold(W1l), fold(W1r)
    be_l = np.einsum("o,otj->j", conv_b, W1l.reshape(COUT, TPOS, H)) + b1l
    be_r = np.einsum("o,otj->j", conv_b, W1r.reshape(COUT, TPOS, H)) + b1r

    perm1 = np.concatenate([np.where(att1 > 0)[0], np.where(att1 <= 0)[0]])
    Fp1 = int((att1 > 0).sum())
    catt1 = np.maximum(np.abs(att1[perm1]), 1e-12)
    perm2 = np.concatenate([np.where(att2 > 0)[0], np.where(att2 <= 0)[0]])
    Fp2 = int((att2 > 0).sum())
    catt2 = np.maximum(np.abs(att2[perm2]), 1e-12)

    flip1 = np.where(np.arange(H) < Fp1, 1.0, -0.2).astype(np.float32)
    V_la = V_l[:, perm1] * (catt1 * flip1)[None, :]
    be_la = be_l[perm1] * catt1 * flip1
    V_ra = V_r[:, perm1] * (catt1 * flip1)[None, :]
    be_ra = be_r[perm1] * catt1 * flip1
    bias1_p = bias1[perm1]
    gamma_p, beta_p = gamma[perm1], beta[perm1]

    flip2 = np.where(np.arange(L) < Fp2, 1.0, -0.2).astype(np.float32)
    W2la = W2l[perm1][:, perm2] * (catt2 * flip2)[None, :]
    W2ra_ = W2r[perm1][:, perm2] * (catt2 * flip2)[None, :]
    b2l_p = b2l[perm2] * catt2 * flip2
    b2ra_p = b2r[perm2] * catt2 * flip2
    bias2_p = bias2[perm2]

    # edges (+ self loops), shard by dst core, sort by dst, block-pad
    loops = np.arange(N, dtype=np.int64)
    src = np.concatenate([ei[0], loops])
    dst = np.concatenate([ei[1], loops])
    per_core = []
    TB = 1
    for c in range(NCORES):
        m = (dst // NLOC) == c
        s_c, d_c = src[m], dst[m] - c * NLOC
        o = np.argsort(d_c, kind="stable")
        s_c, d_c = s_c[o], d_c[o]
        blocks = []
        for b in range(NB):
            bm = (d_c // 128) == b
            blocks.append((s_c[bm], d_c[bm] % 128))
            TB = max(TB, (len(blocks[-1][0]) + 127) // 128)
        per_core.append(blocks)

    NI = TB * 128
    TBS1 = int(os.environ.get("KERNEL_TBS1", 0))
    TBS2 = int(os.environ.get("KERNEL_TBS2", 0))
    TBS1 = min(max(TBS1, 0), TB)
    TBS2 = min(max(TBS2, 0), TB)
    TBP1, TBP2 = TB - TBS1, TB - TBS2

    def wrap16(idx, ni):
        # dma_gather idx layout: idx i at partition i%16, col i//16; x8 groups
        w = idx.reshape(ni // 16, 16).T
        return np.tile(w, (8, 1)).astype(np.int16)

    import ml_dtypes
    core_edges = []
    for c in range(NCORES):
        isrc_a = np.zeros((128, NB, NI // 16), dtype=np.int16)
        idst_a = np.zeros((128, NB, max(TBS1, 1) * 8), dtype=np.int16)
        f8 = ml_dtypes.float8_e4m3
        sab_a = np.zeros((128, NB, TB, 128), dtype=f8)
        sabT1_a = np.zeros((128, NB, max(TBP1, 1), 128), dtype=f8)
        sabT2_a = np.zeros((128, NB, max(TBP2, 1), 128), dtype=f8)
        for b in range(NB):
            s_b, r_b = per_core[c][b]
            n = len(s_b)
            sg = np.zeros(NI, dtype=np.int16)
            dl = np.zeros(NI, dtype=np.int16)
            dr = np.full(NI, 300.0, dtype=np.float32)
            kk, rr = s_b // NLOC, s_b % NLOC
            rpc_h = NLOC // AG_CHUNKS
            sperm = (rr // rpc_h) * (NCORES * rpc_h) + kk * rpc_h + (rr % rpc_h)
            sg[:n] = sperm.astype(np.int16)
            dl[:n] = (r_b + b * 128).astype(np.int16)
            dr[:n] = r_b.astype(np.float32)
            isrc_a[:, b, :] = wrap16(sg, NI)
            if TBS1 > 0:
                idst_a[:, b, :] = wrap16(dl[:TBS1 * 128], TBS1 * 128)
            drm = dr.reshape(TB, 128).T              # [p, t]
            onehot = (drm[:, :, None] ==
                      np.arange(128)[None, None, :]).astype(ml_dtypes.float8_e4m3)
            sab_a[:, b, :, :] = onehot
            if TBP1 > 0:
                sabT1_a[:, b, :, :] = onehot[:, TBS1:, :].transpose(2, 1, 0)
            if TBP2 > 0:
                sabT2_a[:, b, :, :] = onehot[:, TBS2:, :].transpose(2, 1, 0)
        core_edges.append((isrc_a, idst_a, sab_a, sabT1_a, sabT2_a))

    # per-core dense inputs
    flat = x.reshape(N, F0)
    in_maps = []
    for c in range(NCORES):
        fl = np.zeros((NPAD, F0), dtype=np.float32)
        fl[:NLOC] = flat[c * NLOC:(c + 1) * NLOC]
        xT_dev = np.ascontiguousarray(fl.T.reshape(KC0, 128, NPAD).transpose(1, 0, 2))
        isrc_a, idst_a, sab_a, sabT1_a, sabT2_a = core_edges[c]
        maskl_a = (np.arange(128) < (NLOC - (NB - 1) * 128)).astype(np.float32)[:, None]
        im = {
            "xT": _bf16(xT_dev),
            "Wl": _bf16(V_la.reshape(KC0, 128, H).transpose(1, 0, 2)),
            "Wra": _bf16(V_ra.reshape(KC0, 128, H).transpose(1, 0, 2)),
            "beff_l": _bf16(be_la[None, :]),
            "beff_ra": _bf16(be_ra[None, :]),
            "bias1b": _bf16(np.tile(bias1_p, (128, 1))),
            "rcatt1": _bf16(np.tile(1.0 / (catt1 * flip1), (128, 1))),
            "W2l": _bf16(W2la.reshape(KC1, 128, L).transpose(1, 0, 2)),
            "W2ra": _bf16(W2ra_.reshape(KC1, 128, L).transpose(1, 0, 2)),
            "b2l": _bf16(b2l_p[None, :]),
            "b2ra": _bf16(b2ra_p[None, :]),
            "rcatt2": _bf16(np.tile(1.0 / (catt2 * flip2), (128, 1))),
            "bias2b": np.tile(bias2_p, (128, 1)).astype(np.float32),
            "gamma_c": np.ascontiguousarray(gamma_p.reshape(KC1, 128).T).astype(np.float32),
            "beta_c": np.ascontiguousarray(beta_p.reshape(KC1, 128).T).astype(np.float32),
            "maskl": maskl_a.astype(np.float32),
            "ident": _bf16(np.eye(128, dtype=np.float32)),
            "isrc16": isrc_a,
            "idst16": idst_a,
            "sabase": sab_a,
            "sabT1": sabT1_a,
            "sabT2": sabT2_a,
        }
        in_maps.append(im)
    return in_maps, TB, TBS1, TBS2, Fp1, Fp2, perm2


LAST_RESULTS = None


def kernel(**inputs):
    global LAST_RESULTS
    in_maps, TB, TBS1, TBS2, Fp1, Fp2, perm2 = _prep_host(inputs)
    key = (TB, TBS1, TBS2, Fp1, Fp2)
    if key not in _cache:
        _cache[key] = build_program(TB, TBS1, TBS2, Fp1, Fp2)
    nc = _cache[key]
    res = run_bass_kernel_spmd(nc, in_maps, list(range(NCORES)))
    LAST_RESULTS = res
    outs = [np.asarray(res.results[c]["mu"], dtype=np.float32)[:NLOC]
            for c in range(NCORES)]
    mu_dev = np.concatenate(outs, axis=0)
    mu = np.empty_like(mu_dev)
    mu[:, perm2] = mu_dev
    return mu


# revision 16
# speedup vs baseline: 1.0525x; 1.0525x over previous
"""Trainium2 Bass kernel for MatrixGATVAE (2-layer GATv2 encoder), 8-core SPMD.

kernel(**inputs): FULL numpy inputs -> FULL [20000, 128] f32 output.
Sharding: nodes + in-edges by destination across 8 cores; weights replicated;
src-side tables all-gathered; per-block batched dma_gather for src edge rows.

Edge stage (v2):
- e = att.leaky_relu(z) computed directly: w = Lrelu(z, 0.2) on ACT, then two
  even-aligned range reduces (+ odd-Fp straggler column fix) on DVE.
- dst-side rows (z = ga + xra[dst]): per-block 128 unique rows; first TBS
  sub-blocks use dma_gather (POOL), the rest are expanded on PE via the
  transposed one-hot (SaT x XRB -> PSUM) and added to ga on DVE.
- Aggregation: one-hot scatter matmuls (Sa scaled by AA on ACT) as before.
- Conv1d + flatten folded into projection weights; tables pre-scaled by |att|;
  BatchNorm folded into layer-2 projection weights on device.
"""

import os
import sys

import numpy as np

sys.path.insert(0, "/opt/trn_rl_repo")

import concourse.bass as bass
import concourse.bacc as bacc
import concourse.mybir as mybir
import concourse.tile as tile
from concourse.bass_utils import run_bass_kernel_spmd

BF16 = mybir.dt.bfloat16
F32 = mybir.dt.float32
FP8 = mybir.dt.float8e4
I16 = mybir.dt.int16
AF = mybir.ActivationFunctionType
ALU = mybir.AluOpType
AX = mybir.AxisListType

N = 20000
NCORES = 8
NLOC = N // NCORES            # 2500
NPAD = 2560
NB = NPAD // 128              # 20
KW, TPOS, COUT = 24, 32, 64
F0 = 768
H = 512
L = 128
KC0 = F0 // 128               # 6
KC1 = H // 128                # 4
BN_EPS = 1e-5
NQUEUES = 2
AG_CHUNKS = 1

_cache = {}


def _bf16(a):
    import ml_dtypes
    return np.ascontiguousarray(np.asarray(a, dtype=np.float32)).astype(ml_dtypes.bfloat16)


def build_program(TB, TBS1, TBS2, Fp1, Fp2):
    NI = TB * 128
    TBP1, TBP2 = TB - TBS1, TB - TBS2
    nc = bacc.Bacc(num_devices=NCORES, num_swdge_queues=NQUEUES)

    def dp(name, shape, dtype, isOutput=False):
        return nc.declare_dram_parameter(name, shape, dtype, isOutput)

    xT = dp("xT", [128, KC0, NPAD], BF16)
    Wl = dp("Wl", [128, KC0, H], BF16)
    Wra = dp("Wra", [128, KC0, H], BF16)
    beff_l = dp("beff_l", [1, H], BF16)
    beff_ra = dp("beff_ra", [1, H], BF16)
    bias1b = dp("bias1b", [128, H], BF16)
    rcatt1 = dp("rcatt1", [128, H], BF16)
    W2l = dp("W2l", [128, KC1, L], BF16)
    W2ra = dp("W2ra", [128, KC1, L], BF16)
    b2l = dp("b2l", [1, L], BF16)
    b2ra = dp("b2ra", [1, L], BF16)
    rcatt2 = dp("rcatt2", [128, L], BF16)
    bias2b = dp("bias2b", [128, L], F32)
    gamma_c = dp("gamma_c", [128, KC1], F32)
    beta_c = dp("beta_c", [128, KC1], F32)
    maskl = dp("maskl", [128, 1], F32)
    ident = dp("ident", [128, 128], BF16)
    isrc16 = dp("isrc16", [128, NB, NI // 16], I16)
    idst16 = dp("idst16", [128, NB, max(TBS1, 1) * 8], I16)
    sabase = dp("sabase", [128, NB, TB, 128], FP8)
    sabT1 = dp("sabT1", [128, NB, max(TBP1, 1), 128], FP8)
    sabT2 = dp("sabT2", [128, NB, max(TBP2, 1), 128], FP8)
    mu_out = dp("mu", [NPAD, L], F32, isOutput=True)

    XLA_loc = nc.dram_tensor("XLA_loc", [NPAD, H], BF16)
    XLAg = nc.dram_tensor("XLAg", [N, H], BF16, addr_space="Shared")
    XRAT1 = nc.dram_tensor("XRAT1", [NPAD, H], BF16)
    H1Dc = [nc.dram_tensor(f"H1Dc{c}", [NPAD, 128], BF16) for c in range(KC1)]
    XLA2_loc = nc.dram_tensor("XLA2_loc", [NPAD, L], BF16)
    XLA2g = nc.dram_tensor("XLA2g", [N, L], BF16, addr_space="Shared")
    XRAT2 = nc.dram_tensor("XRAT2", [NPAD, L], BF16)
    ST_loc = nc.dram_tensor("ST_loc", [1, 2 * H], F32)
    ST_red = nc.dram_tensor("ST_red", [1, 2 * H], F32, addr_space="Shared")

    grp = [list(range(NCORES))]

    with tile.TileContext(nc) as tc:
        with (
            tc.tile_pool(name="const", bufs=1) as cpool,
            tc.tile_pool(name="zsrc", bufs=3) as zs_pool,
            tc.tile_pool(name="zdst", bufs=2) as zd_pool,
            tc.tile_pool(name="lw", bufs=1) as w_pool,
            tc.tile_pool(name="sab", bufs=2) as sab_pool,
            tc.tile_pool(name="work", bufs=3) as work,
            tc.tile_pool(name="blk", bufs=2) as blk,
            tc.tile_pool(name="acc", bufs=2, space="PSUM") as ps,
            tc.tile_pool(name="psS", bufs=1, space="PSUM") as ps_s,
            tc.tile_pool(name="psZ", bufs=2, space="PSUM") as ps_z,
        ):
            def load(tag, dram):
                t = cpool.tile(dram.shape, dram.dtype, tag=tag)
                nc.sync.dma_start(t[:], dram[:])
                return t

            Wl_s = load("Wl", Wl)
            Wra_s = load("Wra", Wra)
            beffl_s = load("beffl", beff_l)
            beffra_s = load("beffra", beff_ra)
            bias1b_s = load("bias1b", bias1b)
            rcatt1_s = load("rcatt1", rcatt1)
            W2l_s = load("W2l", W2l)
            W2ra_s = load("W2ra", W2ra)
            b2l_s = load("b2l", b2l)
            b2ra_s = load("b2ra", b2ra)
            rcatt2_s = load("rcatt2", rcatt2)
            bias2b_s = load("bias2b", bias2b)
            gamma_s = load("gamma", gamma_c)
            beta_s = load("beta", beta_c)
            maskl_s = load("maskl", maskl)
            ident_s = load("ident", ident)
            idst_s = load("idst16", idst16)

            ones_col = cpool.tile([128, 1], BF16, tag="ones_col")
            nc.vector.memset(ones_col[:], 1.0)
            one_row = cpool.tile([1, 128], BF16, tag="one_row")
            nc.vector.memset(one_row[:], 1.0)

            W2fl = cpool.tile([128, KC1, L], BF16, tag="W2fl")
            W2fra = cpool.tile([128, KC1, L], BF16, tag="W2fra")
            b2e = cpool.tile([1, L], BF16, tag="b2e")
            b2era = cpool.tile([1, L], BF16, tag="b2era")

            # ============ layer-1 projections -> tables ============
            # AllGather is chunked so transfers overlap the projection loop.
            assert NB % AG_CHUNKS == 0 and NLOC % AG_CHUNKS == 0
            bpc = NB // AG_CHUNKS
            rpc = NLOC // AG_CHUNKS
            for b in range(NB):
                sl = slice(b * 128, (b + 1) * 128)
                pl = ps.tile([128, H], F32, tag="acc")
                pra = ps.tile([128, H], F32, tag="acc")
                xTb = blk.tile([128, KC0, 128], BF16, tag="xTb")
                nc.sync.dma_start(xTb[:], xT[:, :, sl])
                for c in range(KC0):
                    lhsT = xTb[:, c, :]
                    nc.tensor.matmul(pl[:], lhsT, Wl_s[:, c, :], start=(c == 0), stop=False)
                    nc.tensor.matmul(pra[:], lhsT, Wra_s[:, c, :], start=(c == 0), stop=False)
                nc.tensor.matmul(pl[:], one_row[:1, :], beffl_s[:1, :], start=False, stop=True)
                nc.tensor.matmul(pra[:], one_row[:1, :], beffra_s[:1, :], start=False, stop=True)
                xl_sb = blk.tile([128, H], BF16, tag="xl")
                xra_sb = blk.tile([128, H], BF16, tag="xra")
                nc.scalar.activation(xl_sb[:], pl[:], AF.Copy)
                nc.scalar.activation(xra_sb[:], pra[:], AF.Copy)
                nc.sync.dma_start(XLA_loc[sl, :], xl_sb[:])
                nc.sync.dma_start(XRAT1[sl, :], xra_sb[:])
                if b % bpc == bpc - 1:
                    cchunk = b // bpc
                    rsl = slice(cchunk * rpc, (cchunk + 1) * rpc)
                    nc.gpsimd.collective_compute(
                        "AllGather", ALU.bypass, replica_groups=grp,
                        ins=[XLA_loc[rsl, :]],
                        outs=[XLAg[cchunk * NCORES * rpc:(cchunk + 1) * NCORES * rpc, :]])

            # ============ fused edge stage (logits + aggregation) ============
            # Software-pipelined; BPG blocks share one gather instruction to
            # amortize SWDGE fixed + completion overhead.
            def edge_stage(W, Fp, TBLg, XRAT_d, sabT, lay, post, BPG):
                NBG = NB // BPG
                TBG = TB * BPG
                def logit_phase(g):
                    isx = blk.tile([128, BPG * NI // 16], I16, tag="isx")
                    nc.sync.dma_start(
                        isx[:], isrc16[:, g * BPG:(g + 1) * BPG, :].rearrange(
                            "p b i -> p (b i)"))
                    ga = zs_pool.tile([128, TBG, W], BF16, tag="ga")
                    nc.gpsimd.dma_gather(
                        out_ap=ga[:, :, :], in_ap=TBLg[:, :], idxs_ap=isx[:, :],
                        num_idxs=BPG * NI, num_idxs_reg=BPG * NI, elem_size=W,
                        queue_num=0, single_packet=False)
                    z = zd_pool.tile([128, TBG, W], BF16, tag="z")
                    xrbs = []
                    for sb in range(BPG):
                        xrb = work.tile([128, W], BF16, tag=f"xrb{sb}")
                        nc.sync.dma_start(
                            xrb[:],
                            XRAT_d[(g * BPG + sb) * 128:(g * BPG + sb + 1) * 128, :])
                        xrbs.append(xrb)
                    sabT_s = sab_pool.tile([128, BPG, TB, 128], FP8, tag="sabT")
                    nc.sync.dma_start(sabT_s[:], sabT[:, g * BPG:(g + 1) * BPG, :, :])
                    w = w_pool.tile([128, TBG, W], BF16, tag="lw")
                    ee = work.tile([128, TBG], F32, tag=f"ee{lay}")
                    HG = (TBG + 1) // 2
                    for half in range(2):
                        t0, t1 = half * HG, min((half + 1) * HG, TBG)
                        t = t0
                        while t < t1:
                            tw = min(2, t1 - t)
                            pz = ps_z.tile([128, 2, W], F32, tag="pz")
                            for j in range(tw):
                                tt = t + j
                                nc.tensor.matmul(pz[:, j, :],
                                                 sabT_s[:, tt // TB, tt % TB, :],
                                                 xrbs[tt // TB][:],
                                                 start=True, stop=False)
                                nc.tensor.matmul(pz[:, j, :], ident_s[:],
                                                 ga[:, tt, :], start=False, stop=True)
                            nc.scalar.activation(z[:, t:t + tw, :], pz[:, 0:tw, :], AF.Copy)
                            t += tw
                        # exact leaky on this half: pos cols max(z,.2z);
                        # neg cols (tables pre-scaled by -0.2) min(z,5z)
                        nc.vector.scalar_tensor_tensor(
                            w[:, t0:t1, 0:Fp], z[:, t0:t1, 0:Fp], 0.2,
                            z[:, t0:t1, 0:Fp], ALU.mult, ALU.max)
                        nc.vector.scalar_tensor_tensor(
                            w[:, t0:t1, Fp:W], z[:, t0:t1, Fp:W], 5.0,
                            z[:, t0:t1, Fp:W], ALU.mult, ALU.min)
                        nc.vector.tensor_reduce(
                            ee[:, t0:t1], w[:, t0:t1, :], axis=AX.X, op=ALU.add)
                    SaB = sab_pool.tile([128, BPG, TB, 128], FP8, tag="SaB")
                    nc.sync.dma_start(SaB[:], sabase[:, g * BPG:(g + 1) * BPG, :, :])
                    return (g, ga, z, ee, SaB)

                def scatter_phase(st):
                    g, ga, z, ee, SaB = st
                    AA = work.tile([128, TBG], F32, tag=f"AA{lay}")
                    nc.scalar.activation(AA[:], ee[:], AF.Exp)
                    AAb = work.tile([128, TBG], BF16, tag=f"AAb{lay}")
                    nc.scalar.activation(AAb[:], AA[:], AF.Copy)
                    for sb in range(BPG):
                        pU = ps.tile([128, H], F32, tag="acc")
                        pS = ps_s.tile([128, 1], F32, tag="pS")
                        for k in range(TB):
                            t = sb * TB + k
                            if t % 2 == 0:
                                nc.vector.tensor_scalar(
                                    z[:, t, :], ga[:, t, :], AA[:, t:t + 1],
                                    None, ALU.mult)
                            else:
                                nc.scalar.activation(
                                    z[:, t, :], ga[:, t, :], AF.Copy,
                                    scale=AA[:, t:t + 1])
                            nc.tensor.matmul(
                                pU[:, 0:W], SaB[:, sb, k, :], z[:, t, :],
                                start=(k == 0), stop=(k == TB - 1))
                            nc.tensor.matmul(
                                pS[:], SaB[:, sb, k, :], AAb[:, t:t + 1],
                                start=(k == 0), stop=(k == TB - 1))
                        post(g * BPG + sb, pU, pS)

                prev = None
                for g in range(NBG):
                    cur = logit_phase(g)
                    if prev is not None:
                        scatter_phase(prev)
                    prev = cur
                scatter_phase(prev)

            # ---- layer-1 edge stage -> h1 ----
            st_acc = cpool.tile([1, 2 * H], F32, tag="st_acc")
            nc.vector.memset(st_acc[:], 0.0)

            def post1(b, pU, pS):
                s_sb = blk.tile([128, 1], F32, tag="s1")
                nc.vector.tensor_scalar(s_sb[:], pS[:], 1e-16, None, ALU.add)
                r_sb = blk.tile([128, 1], F32, tag="r1")
                nc.vector.reciprocal(r_sb[:], s_sb[:])
                hsq = work.tile([128, 2, H], BF16, tag="hsq")
                h1 = hsq[:, 0, :]
                nc.vector.scalar_tensor_tensor(
                    h1[:], pU[:, 0:H], r_sb[:], rcatt1_s[:], ALU.mult, ALU.mult)
                nc.vector.tensor_tensor(h1[:], h1[:], bias1b_s[:], ALU.add)
                nc.vector.tensor_scalar(h1[:], h1[:], 0.0, None, ALU.max)
                if b == NB - 1:
                    nc.vector.tensor_scalar(h1[:], h1[:], maskl_s[:], None, ALU.mult)
                nc.vector.tensor_tensor(hsq[:, 1, :], h1[:], h1[:], ALU.mult)
                stp = ps_z.tile([1, 2 * H], F32, tag="pz")
                nc.tensor.matmul(stp[:, 0:H], ones_col[:], hsq[:, 0, :],
                                 start=True, stop=True)
                nc.tensor.matmul(stp[:, H:2 * H], ones_col[:], hsq[:, 1, :],
                                 start=True, stop=True)
                sts = blk.tile([1, 2 * H], F32, tag="sts")
                nc.scalar.activation(sts[:], stp[:], AF.Copy)
                nc.vector.tensor_tensor(st_acc[:], st_acc[:], sts[:], ALU.add)
                for c in range(KC1):
                    nc.sync.dma_start(
                        H1Dc[c][b * 128:(b + 1) * 128, :],
                        h1[:, c * 128:(c + 1) * 128])

            edge_stage(H, Fp1, XLAg, XRAT1, sabT1, 1, post1, BPG=1)

            # ---- h1T via DMA transpose; BN stats from h1T ----
            h1T = cpool.tile([128, KC1, NPAD], BF16, tag="h1T")
            for c in range(KC1):
                nc.sync.dma_start(h1T[:, c, :], H1Dc[c][:, :], transpose=True)

            nc.sync.dma_start(ST_loc[:, :], st_acc[:])
            nc.gpsimd.collective_compute(
                "AllReduce", ALU.add, replica_groups=grp,
                ins=[ST_loc[:, :]], outs=[ST_red[:, :]])
            str_sb = cpool.tile([128, 2, KC1], F32, tag="str_sb")
            nc.sync.dma_start(
                str_sb[:],
                ST_red[:, :].rearrange("o (a c p) -> (o p) a c", a=2, p=128))

            # ---- BN fold into layer-2 weights ----
            mean = cpool.tile([128, KC1], F32, tag="mean")
            var = cpool.tile([128, KC1], F32, tag="var")
            nc.vector.tensor_scalar(mean[:], str_sb[:, 0, :], 1.0 / N, None, ALU.mult)
            nc.vector.tensor_scalar(var[:], str_sb[:, 1, :], 1.0 / N, None, ALU.mult)
            m2 = cpool.tile([128, KC1], F32, tag="m2")
            nc.vector.tensor_tensor(m2[:], mean[:], mean[:], ALU.mult)
            nc.vector.tensor_tensor(var[:], var[:], m2[:], ALU.subtract)
            nc.vector.tensor_scalar(var[:], var[:], BN_EPS, None, ALU.add)
            sd = cpool.tile([128, KC1], F32, tag="sd")
            nc.scalar.activation(sd[:], var[:], AF.Sqrt)
            rsd = cpool.tile([128, KC1], F32, tag="rsd")
            nc.vector.reciprocal(rsd[:], sd[:])
            tmpn = cpool.tile([128, KC1], F32, tag="tmpn")
            nc.vector.tensor_tensor(tmpn[:], var[:], rsd[:], ALU.mult)
            nc.vector.tensor_tensor(sd[:], sd[:], tmpn[:], ALU.add)
            nc.vector.tensor_scalar(sd[:], sd[:], 0.5, None, ALU.mult)
            nc.vector.reciprocal(rsd[:], sd[:])
            scale = cpool.tile([128, KC1], F32, tag="scale")
            nc.vector.tensor_tensor(scale[:], gamma_s[:], rsd[:], ALU.mult)
            shift = cpool.tile([128, KC1], F32, tag="shift")
            nc.vector.tensor_tensor(shift[:], mean[:], scale[:], ALU.mult)
            nc.vector.tensor_tensor(shift[:], beta_s[:], shift[:], ALU.subtract)
            shift_bf = cpool.tile([128, KC1], BF16, tag="shift_bf")
            nc.vector.tensor_copy(shift_bf[:], shift[:])

            for c in range(KC1):
                nc.vector.tensor_scalar(
                    W2fl[:, c, :], W2l_s[:, c, :], scale[:, c:c + 1], None, ALU.mult)
                nc.vector.tensor_scalar(
                    W2fra[:, c, :], W2ra_s[:, c, :], scale[:, c:c + 1], None, ALU.mult)
            pb = ps.tile([128, H], F32, tag="acc")
            pbra = ps.tile([128, H], F32, tag="acc")
            for c in range(KC1):
                nc.tensor.matmul(pb[0:1, 0:L], shift_bf[:, c:c + 1], W2l_s[:, c, :],
                                 start=(c == 0), stop=False)
                nc.tensor.matmul(pbra[0:1, 0:L], shift_bf[:, c:c + 1], W2ra_s[:, c, :],
                                 start=(c == 0), stop=False)
            nc.tensor.matmul(pb[0:1, 0:L], one_row[:1, 0:1], b2l_s[:1, :], start=False, stop=True)
            nc.tensor.matmul(pbra[0:1, 0:L], one_row[:1, 0:1], b2ra_s[:1, :], start=False, stop=True)
            nc.vector.tensor_copy(b2e[:], pb[0:1, 0:L])
            nc.vector.tensor_copy(b2era[:], pbra[0:1, 0:L])

            # ---- layer-2 projections -> tables ----
            for b in range(NB):
                sl = slice(b * 128, (b + 1) * 128)
                p2 = ps.tile([128, H], F32, tag="acc")
                p2ra = ps.tile([128, H], F32, tag="acc")
                for c in range(KC1):
                    lhsT = h1T[:, c, sl]
                    nc.tensor.matmul(p2[:, 0:L], lhsT, W2fl[:, c, :], start=(c == 0), stop=False)
                    nc.tensor.matmul(p2ra[:, 0:L], lhsT, W2fra[:, c, :], start=(c == 0), stop=False)
                nc.tensor.matmul(p2[:, 0:L], one_row[:1, :], b2e[:1, :], start=False, stop=True)
                nc.tensor.matmul(p2ra[:, 0:L], one_row[:1, :], b2era[:1, :], start=False, stop=True)
                xl2_sb = blk.tile([128, L], BF16, tag="xl2")
                xra2_sb = blk.tile([128, L], BF16, tag="xra2")
                nc.scalar.activation(xl2_sb[:], p2[:, 0:L], AF.Copy)
                nc.scalar.activation(xra2_sb[:], p2ra[:, 0:L], AF.Copy)
                nc.sync.dma_start(XLA2_loc[sl, :], xl2_sb[:])
                nc.sync.dma_start(XRAT2[sl, :], xra2_sb[:])
                if b % bpc == bpc - 1:
                    cchunk = b // bpc
                    rsl = slice(cchunk * rpc, (cchunk + 1) * rpc)
                    nc.gpsimd.collective_compute(
                        "AllGather", ALU.bypass, replica_groups=grp,
                        ins=[XLA2_loc[rsl, :]],
                        outs=[XLA2g[cchunk * NCORES * rpc:(cchunk + 1) * NCORES * rpc, :]])

            # ---- layer-2 edge stage -> mu ----
            def post2(b, pU, pS):
                s_sb = blk.tile([128, 1], F32, tag="s2")
                nc.vector.tensor_scalar(s_sb[:], pS[:], 1e-16, None, ALU.add)
                r_sb = blk.tile([128, 1], F32, tag="r2")
                nc.vector.reciprocal(r_sb[:], s_sb[:])
                mu_sb = blk.tile([128, L], F32, tag="mu")
                nc.vector.scalar_tensor_tensor(
                    mu_sb[:], pU[:, 0:L], r_sb[:], rcatt2_s[:], ALU.mult, ALU.mult)
                nc.vector.tensor_tensor(mu_sb[:], mu_sb[:], bias2b_s[:], ALU.add)
                nc.sync.dma_start(mu_out[b * 128:(b + 1) * 128, :], mu_sb[:])

            edge_stage(L, Fp2, XLA2g, XRAT2, sabT2, 2, post2, BPG=2)

    nc.compile()
    return nc


def _prep_host(inputs):
    x = np.asarray(inputs["x"], dtype=np.float32)
    ei = np.asarray(inputs["edge_index"], dtype=np.int64)
    conv_w = np.asarray(inputs["conv_w"], dtype=np.float32)
    conv_b = np.asarray(inputs["conv_b"], dtype=np.float32)
    W1l = np.asarray(inputs["W1l"], dtype=np.float32)
    b1l = np.asarray(inputs["b1l"], dtype=np.float32)
    W1r = np.asarray(inputs["W1r"], dtype=np.float32)
    b1r = np.asarray(inputs["b1r"], dtype=np.float32)
    att1 = np.asarray(inputs["att1"], dtype=np.float32)
    bias1 = np.asarray(inputs["bias1"], dtype=np.float32)
    gamma = np.asarray(inputs["gamma"], dtype=np.float32)
    beta = np.asarray(inputs["beta"], dtype=np.float32)
    W2l = np.asarray(inputs["W2l"], dtype=np.float32)
    b2l = np.asarray(inputs["b2l"], dtype=np.float32)
    W2r = np.asarray(inputs["W2r"], dtype=np.float32)
    b2r = np.asarray(inputs["b2r"], dtype=np.float32)
    att2 = np.asarray(inputs["att2"], dtype=np.float32)
    bias2 = np.asarray(inputs["bias2"], dtype=np.float32)

    # conv fold: V[(k*32+t), j] = sum_o w[o,k] W[o*32+t, j]
    def fold(W):
        return np.einsum("ok,otj->ktj", conv_w,
                         W.reshape(COUT, TPOS, -1)).reshape(F0, -1)

    V_l, V_r = fold(W1l), fold(W1r)
    be_l = np.einsum("o,otj->j", conv_b, W1l.reshape(COUT, TPOS, H)) + b1l
    be_r = np.einsum("o,otj->j", conv_b, W1r.reshape(COUT, TPOS, H)) + b1r

    perm1 = np.concatenate([np.where(att1 > 0)[0], np.where(att1 <= 0)[0]])
    Fp1 = int((att1 > 0).sum())
    catt1 = np.maximum(np.abs(att1[perm1]), 1e-12)
    perm2 = np.concatenate([np.where(att2 > 0)[0], np.where(att2 <= 0)[0]])
    Fp2 = int((att2 > 0).sum())
    catt2 = np.maximum(np.abs(att2[perm2]), 1e-12)

    flip1 = np.where(np.arange(H) < Fp1, 1.0, -0.2).astype(np.float32)
    V_la = V_l[:, perm1] * (catt1 * flip1)[None, :]
    be_la = be_l[perm1] * catt1 * flip1
    V_ra = V_r[:, perm1] * (catt1 * flip1)[None, :]
    be_ra = be_r[perm1] * catt1 * flip1
    bias1_p = bias1[perm1]
    gamma_p, beta_p = gamma[perm1], beta[perm1]

    flip2 = np.where(np.arange(L) < Fp2, 1.0, -0.2).astype(np.float32)
    W2la = W2l[perm1][:, perm2] * (catt2 * flip2)[None, :]
    W2ra_ = W2r[perm1][:, perm2] * (catt2 * flip2)[None, :]
    b2l_p = b2l[perm2] * catt2 * flip2
    b2ra_p = b2r[perm2] * catt2 * flip2
    bias2_p = bias2[perm2]

    # edges (+ self loops), shard by dst core, sort by dst, block-pad
    loops = np.arange(N, dtype=np.int64)
    src = np.concatenate([ei[0], loops])
    dst = np.concatenate([ei[1], loops])
    per_core = []
    TB = 1
    for c in range(NCORES):
        m = (dst // NLOC) == c
        s_c, d_c = src[m], dst[m] - c * NLOC
        o = np.argsort(d_c, kind="stable")
        s_c, d_c = s_c[o], d_c[o]
        blocks = []
        for b in range(NB):
            bm = (d_c // 128) == b
            blocks.append((s_c[bm], d_c[bm] % 128))
            TB = max(TB, (len(blocks[-1][0]) + 127) // 128)
        per_core.append(blocks)

    NI = TB * 128
    TBS1 = int(os.environ.get("KERNEL_TBS1", 0))
    TBS2 = int(os.environ.get("KERNEL_TBS2", 0))
    TBS1 = min(max(TBS1, 0), TB)
    TBS2 = min(max(TBS2, 0), TB)
    TBP1, TBP2 = TB - TBS1, TB - TBS2

    def wrap16(idx, ni):
        # dma_gather idx layout: idx i at partition i%16, col i//16; x8 groups
        w = idx.reshape(ni // 16, 16).T
        return np.tile(w, (8, 1)).astype(np.int16)

    import ml_dtypes
    core_edges = []
    for c in range(NCORES):
        isrc_a = np.zeros((128, NB, NI // 16), dtype=np.int16)
        idst_a = np.zeros((128, NB, max(TBS1, 1) * 8), dtype=np.int16)
        f8 = ml_dtypes.float8_e4m3
        sab_a = np.zeros((128, NB, TB, 128), dtype=f8)
        sabT1_a = np.zeros((128, NB, max(TBP1, 1), 128), dtype=f8)
        sabT2_a = np.zeros((128, NB, max(TBP2, 1), 128), dtype=f8)
        for b in range(NB):
            s_b, r_b = per_core[c][b]
            n = len(s_b)
            sg = np.zeros(NI, dtype=np.int16)
            dl = np.zeros(NI, dtype=np.int16)
            dr = np.full(NI, 300.0, dtype=np.float32)
            kk, rr = s_b // NLOC, s_b % NLOC
            rpc_h = NLOC // AG_CHUNKS
            sperm = (rr // rpc_h) * (NCORES * rpc_h) + kk * rpc_h + (rr % rpc_h)
            sg[:n] = sperm.astype(np.int16)
            dl[:n] = (r_b + b * 128).astype(np.int16)
            dr[:n] = r_b.astype(np.float32)
            isrc_a[:, b, :] = wrap16(sg, NI)
            if TBS1 > 0:
                idst_a[:, b, :] = wrap16(dl[:TBS1 * 128], TBS1 * 128)
            drm = dr.reshape(TB, 128).T              # [p, t]
            onehot = (drm[:, :, None] ==
                      np.arange(128)[None, None, :]).astype(ml_dtypes.float8_e4m3)
            sab_a[:, b, :, :] = onehot
            if TBP1 > 0:
                sabT1_a[:, b, :, :] = onehot[:, TBS1:, :].transpose(2, 1, 0)
            if TBP2 > 0:
                sabT2_a[:, b, :, :] = onehot[:, TBS2:, :].transpose(2, 1, 0)
        core_edges.append((isrc_a, idst_a, sab_a, sabT1_a, sabT2_a))

    # per-core dense inputs
    flat = x.reshape(N, F0)
    in_maps = []
    for c in range(NCORES):
        fl = np.zeros((NPAD, F0), dtype=np.float32)
        fl[:NLOC] = flat[c * NLOC:(c + 1) * NLOC]
        xT_dev = np.ascontiguousarray(fl.T.reshape(KC0, 128, NPAD).transpose(1, 0, 2))
        isrc_a, idst_a, sab_a, sabT1_a, sabT2_a = core_edges[c]
        maskl_a = (np.arange(128) < (NLOC - (NB - 1) * 128)).astype(np.float32)[:, None]
        im = {
            "xT": _bf16(xT_dev),
            "Wl": _bf16(V_la.reshape(KC0, 128, H).transpose(1, 0, 2)),
            "Wra": _bf16(V_ra.reshape(KC0, 128, H).transpose(1, 0, 2)),
            "beff_l": _bf16(be_la[None, :]),
            "beff_ra": _bf16(be_ra[None, :]),
            "bias1b": _bf16(np.tile(bias1_p, (128, 1))),
            "rcatt1": _bf16(np.tile(1.0 / (catt1 * flip1), (128, 1))),
            "W2l": _bf16(W2la.reshape(KC1, 128, L).transpose(1, 0, 2)),
            "W2ra": _bf16(W2ra_.reshape(KC1, 128, L).transpose(1, 0, 2)),
            "b2l": _bf16(b2l_p[None, :]),
            "b2ra": _bf16(b2ra_p[None, :]),
            "rcatt2": _bf16(np.tile(1.0 / (catt2 * flip2), (128, 1))),
            "bias2b": np.tile(bias2_p, (128, 1)).astype(np.float32),
            "gamma_c": np.ascontiguousarray(gamma_p.reshape(KC1, 128).T).astype(np.float32),
            "beta_c": np.ascontiguousarray(beta_p.reshape(KC1, 128).T).astype(np.float32),
            "maskl": maskl_a.astype(np.float32),
            "ident": _bf16(np.eye(128, dtype=np.float32)),
            "isrc16": isrc_a,
            "idst16": idst_a,
            "sabase": sab_a,
            "sabT1": sabT1_a,
            "sabT2": sabT2_a,
        }
        in_maps.append(im)
    return in_maps, TB, TBS1, TBS2, Fp1, Fp2, perm2


LAST_RESULTS = None


def kernel(**inputs):
    global LAST_RESULTS
    in_maps, TB, TBS1, TBS2, Fp1, Fp2, perm2 = _prep_host(inputs)
    key = (TB, TBS1, TBS2, Fp1, Fp2)
    if key not in _cache:
        _cache[key] = build_program(TB, TBS1, TBS2, Fp1, Fp2)
    nc = _cache[key]
    res = run_bass_kernel_spmd(nc, in_maps, list(range(NCORES)))
    LAST_RESULTS = res
    outs = [np.asarray(res.results[c]["mu"], dtype=np.float32)[:NLOC]
            for c in range(NCORES)]
    mu_dev = np.concatenate(outs, axis=0)
    mu = np.empty_like(mu_dev)
    mu[:, perm2] = mu_dev
    return mu
